# revision 7
# baseline (speedup 1.0000x reference)
"""Trainium2 Bass kernel for sliding-window GQA attention block.

Reference computation (B=2, S=4096, DIM=1024, H=16 q-heads, KV=2 kv-heads,
D=64, W=256 window):
    q = x@Wq + bq ; k = x@Wk + bk ; v = x@Wv + bv        (GQA repeat kv x8)
    local attention: query t attends keys [t-128, t+128) (zero-padded edges,
    no 1/sqrt(d) scaling), softmax, out = probs@v
    y = out@Wo + bo

Sharding: 8 cores = batch(2) x seq-quarter(4). Each core computes 1024
query rows end-to-end (all 16 heads) from a 1280-row haloed x slice.
No cross-core communication; host pads/transposes/gathers.

On-device pipeline per core (all matmuls bf16, fp32 PSUM accumulation):
  QKV projections (QK biases folded via per-partition DVE add / K=1
  indicator-row matmuls, which also zero K,V at padded halo rows) ->
  scores computed directly TRANSPOSED (S^T[u,t], keys on partitions;
  both kv-halves of one key-chunk row-packed into a single 2-bank PSUM
  tile so one exp covers them) -> exp on ScalarE (the only ScalarE op;
  all copies live on the DVE) -> 0/1 band-mask multiply on VectorE for
  the two triangular chunks only -> probs @ [V|1] grouped 4 heads per
  PSUM bank (the ones column emits the softmax denominator at column
  64 of each head's 65-col group) -> one strided reciprocal + one
  broadcast tensor_tensor multiply normalizes 4 heads at once ->
  PE-transpose of the 128x128 attn blocks -> out-projection with the
  bias folded via a precomputed broadcast row added during the DVE
  PSUM->SBUF copy. Input DMAs are column-sliced and ordered so the
  K/V-projection data lands first, spread over 4 engine queues, while
  junk identity matmuls warm the PE HAM clock gate during the wait.
"""

import functools
import numpy as np

B, S, DIM = 2, 4096, 1024
H, KV, D = 16, 2, 64
W, HW = 256, 128
NCORES = 8
QT = 4           # sequence quarters
T = S // QT      # 1024 query rows per core
TH = T + 2 * HW  # 1280 haloed rows
XSPLIT = 640     # xT column slab boundary


@functools.lru_cache(maxsize=1)
def _build_nc():
    import concourse.bacc as bacc
    import concourse.tile as tile
    from concourse import mybir
    from concourse.masks import make_identity

    f32 = mybir.dt.float32
    bf16 = mybir.dt.bfloat16
    Exp = mybir.ActivationFunctionType.Exp

    nc = bacc.Bacc("TRN2", target_bir_lowering=False, debug=False)

    xT = nc.dram_tensor("xT", [DIM, TH], bf16, kind="ExternalInput")
    wq = nc.dram_tensor("Wq", [DIM, DIM], bf16, kind="ExternalInput")
    wkv = nc.dram_tensor("Wkv", [DIM, 2 * KV * D], bf16, kind="ExternalInput")
    wo = nc.dram_tensor("Wo", [DIM, DIM], bf16, kind="ExternalInput")
    bqc = nc.dram_tensor("bqc", [128, 8], f32, kind="ExternalInput")
    # [bk (128) | bv (128) | bo (1024) | ind (1280)]
    crow = nc.dram_tensor("crow", [1, 2560], bf16, kind="ExternalInput")
    out = nc.dram_tensor("out", [T, DIM], bf16, kind="ExternalOutput")

    with tile.TileContext(nc) as tc:
        with tc.tile_pool(name="const", bufs=1) as const, \
             tc.tile_pool(name="w", bufs=1) as wpool, \
             tc.tile_pool(name="act", bufs=1) as actp, \
             tc.tile_pool(name="attn", bufs=2) as attnp, \
             tc.tile_pool(name="ps", bufs=2, space="PSUM") as ps:

            # ---- constants first: gpsimd must finish these before it is
            # used as a DMA issue queue, and the PE warmup needs ident ------
            ident = const.tile([128, 128], bf16, tag="ident")
            make_identity(nc, ident)
            # 0/1 window masks, transposed (key r, query c) orientation, for
            # the two triangular chunks; both kv-halves side by side.
            # j=0 chunk: valid where r >= c; j=2 chunk: valid where r < c.
            mask_lo = const.tile([128, 1024], bf16, tag="mask_lo")
            mask_hi = const.tile([128, 1024], bf16, tag="mask_hi")
            nc.gpsimd.memset(mask_lo, 1.0)
            nc.gpsimd.memset(mask_hi, 1.0)
            for blk2 in range(0, 1024, 128):
                nc.gpsimd.affine_select(
                    out=mask_lo[:, blk2:blk2 + 128],
                    in_=mask_lo[:, blk2:blk2 + 128],
                    compare_op=mybir.AluOpType.is_ge,
                    fill=0.0, base=0, pattern=[[-1, 128]],
                    channel_multiplier=1)
                nc.gpsimd.affine_select(
                    out=mask_hi[:, blk2:blk2 + 128],
                    in_=mask_hi[:, blk2:blk2 + 128],
                    compare_op=mybir.AluOpType.is_ge,
                    fill=0.0, base=-1, pattern=[[1, 128]],
                    channel_multiplier=-1)
            ones_row = const.tile([1, 128], bf16, tag="ones")
            nc.vector.memset(ones_row, 1.0)

            # ---- PE warmup: keeps the HAM clock gate open while the input
            # DMAs stream; sized to roughly cover the arrival window --------
            junk_ps = ps.tile([128, 128], f32, tag="proj", name="junk")
            for _ in range(90):
                nc.tensor.matmul(out=junk_ps, lhsT=ident, rhs=ident,
                                 start=True, stop=True)

            # ---- DMAs: few big multi-chunk transfers, explicit queues -----
            bq_sb = const.tile([128, 8], f32, tag="bq")
            crow_sb = const.tile([1, 2560], bf16, tag="crow")
            bkr = crow_sb[:, 0:KV * D]
            bvr = crow_sb[:, KV * D:2 * KV * D]
            bor = crow_sb[:, 256:256 + DIM]
            ind_sb = crow_sb[:, 1280:1280 + TH]

            wkv_big = wpool.tile([128, 8 * 2 * KV * D], bf16, tag="wkv")
            wkv_v = wkv_big.rearrange("p (k c) -> p k c", k=8)
            wk_sb = [wkv_v[:, k, 0:KV * D] for k in range(8)]
            wv_sb = [wkv_v[:, k, KV * D:2 * KV * D] for k in range(8)]
            xT_big = wpool.tile([128, 8 * TH], bf16, tag="xT")
            xT_v = xT_big.rearrange("p (k c) -> p k c", k=8)
            xT_sb = [xT_v[:, k, :] for k in range(8)]
            wq_big = wpool.tile([128, 8 * DIM], bf16, tag="wq")
            wq_v = wq_big.rearrange("p (k c) -> p k c", k=8)
            wq_sb = [wq_v[:, k, :] for k in range(8)]
            wo_big = wpool.tile([128, 8 * DIM], bf16, tag="wo")
            wo_v = wo_big.rearrange("p (k c) -> p k c", k=8)
            wo_sb = [wo_v[:, k, :] for k in range(8)]

            xT_r = xT.rearrange("(k p) c -> p k c", k=8)
            wq_r = wq.rearrange("(k p) c -> p k c", k=8)
            wo_r = wo.rearrange("(k p) c -> p k c", k=8)
            wkv_r = wkv.rearrange("(k p) c -> p k c", k=8)

            # sync queue: first xT slab halves + wq + wo + slab1 (k 0-3)
            nc.sync.dma_start(out=xT_v[:, 0:4, 0:XSPLIT],
                              in_=xT_r[:, 0:4, 0:XSPLIT])
            nc.sync.dma_start(out=wq_v[:, 0:3, :], in_=wq_r[:, 0:3, :])
            nc.sync.dma_start(out=xT_v[:, 0:4, XSPLIT:TH],
                              in_=xT_r[:, 0:4, XSPLIT:TH])
            nc.sync.dma_start(out=wo_v[:, 0:4, :], in_=wo_r[:, 0:4, :])
            # scalar queue: the other halves
            nc.scalar.dma_start(out=xT_v[:, 4:8, 0:XSPLIT],
                                in_=xT_r[:, 4:8, 0:XSPLIT])
            nc.scalar.dma_start(out=wq_v[:, 3:6, :], in_=wq_r[:, 3:6, :])
            nc.scalar.dma_start(out=xT_v[:, 4:8, XSPLIT:TH],
                                in_=xT_r[:, 4:8, XSPLIT:TH])
            nc.scalar.dma_start(out=wo_v[:, 4:8, :], in_=wo_r[:, 4:8, :])
            # gpsimd queue (free after the constant builds above)
            nc.gpsimd.dma_start(out=crow_sb, in_=crow[:, :])
            nc.gpsimd.dma_start(out=bq_sb, in_=bqc[:, :])
            nc.gpsimd.dma_start(out=wkv_v[:, :, :], in_=wkv_r)
            nc.gpsimd.dma_start(out=wq_v[:, 6:8, :], in_=wq_r[:, 6:8, :])

            # bo broadcast to all 128 partitions (K=1 outer product), used
            # by the DVE copy-out add; replaces per-tile K=1 bias matmuls.
            bo_bc = const.tile([128, DIM], bf16, tag="bo_bc")
            for n in range(2):
                bo_ps = ps.tile([128, 512], f32, tag="proj", name="bo_ps")
                nc.tensor.matmul(out=bo_ps, lhsT=ones_row,
                                 rhs=bor[:, n * 512:(n + 1) * 512],
                                 start=True, stop=True)
                nc.vector.tensor_copy(out=bo_bc[:, n * 512:(n + 1) * 512],
                                      in_=bo_ps)

            # ---- K projection over halo; zero at padded rows via ind fold -
            kT_sb = actp.tile([128, TH], bf16, tag="kT")

            def k_proj(c0, cw):
                k_ps = ps.tile([128, 512], f32, tag="proj", name="k_ps")
                for k in range(8):
                    nc.tensor.matmul(
                        out=k_ps[:, :cw], lhsT=wk_sb[k],
                        rhs=xT_sb[k][:, c0:c0 + cw],
                        start=(k == 0), stop=False)
                nc.tensor.matmul(
                    out=k_ps[:, :cw], lhsT=bkr, rhs=ind_sb[:, c0:c0 + cw],
                    start=False, stop=True)
                nc.vector.tensor_copy(out=kT_sb[:, c0:c0 + cw],
                                      in_=k_ps[:, :cw])

            # ---- V projection (keys on partitions). Layout per u-tile is
            # [V_kv0 (64) | 1 | V_kv1 (64) | 1]: the ones column appended to
            # each kv-slice makes the probs@[V|1] matmul emit the softmax
            # denominator as output column 64 for free. ---------------------
            NU = TH // 128
            v_sb = actp.tile([128, NU * 130], bf16, tag="V")
            v_view = v_sb.rearrange("p (u g c) -> p u g c", u=NU, g=2)
            nc.vector.memset(v_view[:, :, :, 64:65], 1.0)

            def v_proj(ut):
                v_ps = ps.tile([128, 512], f32, tag="proj", name="v_ps")
                for k in range(8):
                    nc.tensor.matmul(
                        out=v_ps[:, :KV * D],
                        lhsT=xT_sb[k][:, ut * 128:(ut + 1) * 128],
                        rhs=wv_sb[k], start=(k == 0), stop=False)
                nc.tensor.matmul(
                    out=v_ps[:, :KV * D],
                    lhsT=ind_sb[:, ut * 128:(ut + 1) * 128], rhs=bvr,
                    start=False, stop=True)
                nc.vector.tensor_copy(
                    out=v_view[:, ut, :, 0:64],
                    in_=v_ps[:, :KV * D].rearrange("p (g c) -> p g c", g=2))

            # ---- Q projection: qT tile g holds heads (2g, 2g+1) along the
            # free dim and heads (+8) on the upper partition half ------------
            qT_sb = []
            for g in range(2):
                t_qt = actp.tile([128, 4 * T], bf16, tag=f"qT{g}",
                                 name=f"qT{g}")
                qT_sb.append(t_qt)

            def q_proj(m, n):
                q_ps = ps.tile([128, 512], f32, tag="proj", name="q_ps")
                for k in range(8):
                    nc.tensor.matmul(
                        out=q_ps,
                        lhsT=wq_sb[k][:, m * 128:(m + 1) * 128],
                        rhs=xT_sb[k][:, HW + n * 512: HW + (n + 1) * 512],
                        start=(k == 0), stop=(k == 7))
                off = (m % 4) * T + n * 512
                nc.vector.tensor_scalar_add(
                    out=qT_sb[m // 4][:, off:off + 512], in0=q_ps,
                    scalar1=bq_sb[:, m:m + 1])

            # ---- pre-attention work (data-arrival ordered) -----------------
            k_proj(0, 512)
            k_proj(512, XSPLIT - 512)
            for ut in range(3):
                v_proj(ut)
            for m in range(8):
                q_proj(m, 0)

            # ---- attention + output transpose + (skewed) out-projection ---
            attnT = actp.tile([128, 8 * T], bf16, tag="attnT")
            attnT_v = attnT.rearrange("p (k t) -> p k t", k=8)

            def out_proj(mt):
                out_t = attnp.tile([128, DIM], bf16, tag="outt")
                for n in range(2):
                    o2 = ps.tile([128, 512], f32, tag="proj", name="o2_ps")
                    for k in range(8):
                        nc.tensor.matmul(
                            out=o2,
                            lhsT=attnT[:, k * T + mt * 128:
                                       k * T + (mt + 1) * 128],
                            rhs=wo_sb[k][:, n * 512:(n + 1) * 512],
                            start=(k == 0), stop=(k == 7))
                    nc.vector.tensor_add(
                        out=out_t[:, n * 512:(n + 1) * 512], in0=o2,
                        in1=bo_bc[:, n * 512:(n + 1) * 512])
                    nc.sync.dma_start(
                        out=out[mt * 128:(mt + 1) * 128,
                                n * 512:(n + 1) * 512],
                        in_=out_t[:, n * 512:(n + 1) * 512])

            for mt in range(8):
                # skewed out-projection: early tiles run it after their
                # attention (Wo may still be in flight); late tiles run it
                # first so only out_proj(7) remains after the last attention.
                if mt >= 4:
                    out_proj(mt - 1)
                qcol = mt * 128
                u0 = qcol  # halo col of first attended key
                attn_t = attnp.tile([128, DIM], bf16, tag="attn")
                for gg in range(2):
                    qv = qT_sb[gg].rearrange("p (i t) -> p i t", i=4)
                    p2s = []
                    for j in range(3):
                        # both kv-halves of key-chunk j, row-packed into one
                        # 2-bank PSUM tile; four same-kv heads stream as one
                        # N=512 rhs per half.
                        s2 = ps.tile([128, 1024], f32, tag="s2", bufs=2,
                                     name="s2")
                        for half in range(2):
                            nc.tensor.matmul(
                                out=s2[:, half * 512:(half + 1) * 512],
                                lhsT=kT_sb[half * 64:(half + 1) * 64,
                                           u0 + j * 128:u0 + (j + 1) * 128],
                                rhs=qv[half * 64:(half + 1) * 64, :,
                                       qcol:qcol + 128],
                                start=True, stop=True,
                                tile_position=(64 * half, 0))
                        p2 = attnp.tile([128, 1024], bf16, tag="P", bufs=6,
                                        name="p2")
                        nc.scalar.activation(out=p2, in_=s2, func=Exp)
                        if j == 0:
                            nc.vector.tensor_mul(p2, p2, mask_lo)
                        elif j == 2:
                            nc.vector.tensor_mul(p2, p2, mask_hi)
                        p2s.append(p2)
                    for half in range(2):
                        # 4 heads share one PSUM bank: [a, 0:64]=attn out,
                        # [a, 64]=softmax denominator.
                        o4 = ps.tile([128, 260], f32, tag="o4", bufs=2,
                                     name="o4")
                        o4v = o4.rearrange("p (a c) -> p a c", a=4)
                        for a in range(4):
                            for j in range(3):
                                nc.tensor.matmul(
                                    out=o4v[:, a, :],
                                    lhsT=p2s[j][:, half * 512 + a * 128:
                                                half * 512 + (a + 1) * 128],
                                    rhs=v_view[:, mt + j, half, 0:65],
                                    start=(j == 0), stop=(j == 2))
                        rc4 = attnp.tile([128, 4], f32, tag="rc4", bufs=4,
                                         name="rc4")
                        nc.vector.reciprocal(out=rc4[:, :].unsqueeze(2),
                                             in_=o4v[:, :, 64:65])
                        hbase = (4 * gg + 8 * half) * 64
                        dst = attn_t[:, hbase:hbase + 256].rearrange(
                            "p (a d) -> p a d", a=4)
                        nc.vector.tensor_mul(
                            dst, o4v[:, :, 0:64],
                            rc4[:, :].unsqueeze(2).broadcast_to([128, 4, 64]))
                # transpose attn rows (t) x cols (hd) -> attnT k-tiles
                for g in range(3):
                    kcnt = 3 if g < 2 else 2
                    at_ps = ps.tile([128, 384], bf16, tag="proj", bufs=2,
                                    name="at_ps")
                    for jj in range(kcnt):
                        kk = g * 3 + jj
                        nc.tensor.matmul(
                            out=at_ps[:, jj * 128:(jj + 1) * 128],
                            lhsT=attn_t[:, kk * 128:(kk + 1) * 128],
                            rhs=ident, is_transpose=True,
                            start=(jj == 0), stop=(jj == kcnt - 1))
                    src = at_ps[:, :kcnt * 128].rearrange(
                        "p (j c) -> p j c", j=kcnt)
                    dst = attnT_v[:, g * 3:g * 3 + kcnt, qcol:qcol + 128]
                    nc.vector.tensor_copy(out=dst, in_=src)

                if 1 <= mt <= 3:
                    out_proj(mt - 1)
                # just-in-time projection work keyed to DMA arrival order
                if mt == 0:
                    v_proj(3)
                    v_proj(4)
                    k_proj(XSPLIT, 512)
                elif mt == 1:
                    k_proj(XSPLIT + 512, TH - XSPLIT - 512)
                    v_proj(5)
                elif mt == 2:
                    v_proj(6)
                    for m in range(4):
                        q_proj(m, 1)
                elif mt == 3:
                    v_proj(7)
                    for m in range(4, 8):
                        q_proj(m, 1)
                elif mt == 4:
                    v_proj(8)
                    v_proj(9)
            out_proj(7)

    nc.compile()
    return nc


def _host_prep(x, Wq, bq, Wk, bk, Wv, bv, Wo, bo):
    import ml_dtypes
    bf16 = ml_dtypes.bfloat16

    # permute Wq/bq columns so qT m-tile holds head m on partitions 0-63 and
    # head m+8 on partitions 64-127 (enables row-packed score matmuls)
    idx = np.empty(DIM, dtype=np.int64)
    for m in range(8):
        for j in range(128):
            h = m if j < 64 else m + 8
            idx[m * 128 + j] = h * D + (j % 64)
    wq_p = np.ascontiguousarray(Wq[:, idx]).astype(bf16)
    bq_p = bq[idx].astype(np.float32).reshape(8, 128).T.copy()  # (128, 8)
    wkv_b = np.ascontiguousarray(
        np.concatenate([Wk, Wv], axis=1)).astype(bf16)
    wo_b = np.ascontiguousarray(Wo).astype(bf16)

    in_maps = []
    for c in range(NCORES):
        b, qt = c // QT, c % QT
        lo, hi = qt * T - HW, qt * T + T + HW
        xs = np.zeros((TH, DIM), dtype=np.float32)
        s0, s1 = max(lo, 0), min(hi, S)
        xs[s0 - lo:s1 - lo] = x[b, s0:s1]
        crow = np.zeros((1, 2560), dtype=np.float32)
        crow[0, 0:128] = bk
        crow[0, 128:256] = bv
        crow[0, 256:1280] = bo
        crow[0, 1280 + (s0 - lo):1280 + (s1 - lo)] = 1.0
        in_maps.append({
            "xT": np.ascontiguousarray(xs.T).astype(bf16),
            "Wq": wq_p, "Wkv": wkv_b, "Wo": wo_b,
            "bqc": bq_p, "crow": crow.astype(bf16),
        })
    return in_maps


def kernel(x, Wq, bq, Wk, bk, Wv, bv, Wo, bo):
    from concourse.bass_utils import run_bass_kernel_spmd

    x, Wq, bq, Wk, bk, Wv, bv, Wo, bo = (
        np.asarray(a, dtype=np.float32)
        for a in (x, Wq, bq, Wk, bk, Wv, bv, Wo, bo))
    nc = _build_nc()
    in_maps = _host_prep(x, Wq, bq, Wk, bk, Wv, bv, Wo, bo)
    res = run_bass_kernel_spmd(nc, in_maps, core_ids=list(range(NCORES)))
    out = np.empty((B, S, DIM), dtype=np.float32)
    for c in range(NCORES):
        b, qt = c // QT, c % QT
        out[b, qt * T:(qt + 1) * T] = res.results[c]["out"].astype(np.float32)
    return out


# revision 13
# speedup vs baseline: 1.0610x; 1.0610x over previous
"""Trainium2 Bass kernel for sliding-window GQA attention block.

Reference computation (B=2, S=4096, DIM=1024, H=16 q-heads, KV=2 kv-heads,
D=64, W=256 window):
    q = x@Wq + bq ; k = x@Wk + bk ; v = x@Wv + bv        (GQA repeat kv x8)
    local attention: query t attends keys [t-128, t+128) (zero-padded edges,
    no 1/sqrt(d) scaling), softmax, out = probs@v
    y = out@Wo + bo

Sharding: 8 cores = batch(2) x seq-quarter(4). Each core computes 1024
query rows end-to-end (all 16 heads) from a 1280-row haloed x slice.
No cross-core communication; host pads/transposes/gathers.

On-device pipeline per core (all matmuls bf16, fp32 PSUM accumulation):
  QKV projections (QK biases folded via per-partition DVE add / K=1
  indicator-row matmuls, which also zero K,V at padded halo rows) ->
  scores computed directly TRANSPOSED (S^T[u,t], keys on partitions;
  both kv-halves of one key-chunk row-packed into a single 2-bank PSUM
  tile so one exp covers them) -> exp on ScalarE (the only ScalarE op;
  all copies live on the DVE) -> 0/1 band-mask multiply on VectorE for
  the two triangular chunks only -> probs @ [V|1] grouped 4 heads per
  PSUM bank (the ones column emits the softmax denominator at column
  64 of each head's 65-col group) -> one strided reciprocal + one
  broadcast tensor_tensor multiply normalizes 4 heads at once ->
  PE-transpose of the 128x128 attn blocks -> out-projection with the
  bias folded via a precomputed broadcast row added during the DVE
  PSUM->SBUF copy. Input DMAs are column-sliced and ordered so the
  K/V-projection data lands first, spread over 4 engine queues, while
  junk identity matmuls warm the PE HAM clock gate during the wait.
"""

import functools
import numpy as np

B, S, DIM = 2, 4096, 1024
H, KV, D = 16, 2, 64
W, HW = 256, 128
NCORES = 8
QT = 4           # sequence quarters
T = S // QT      # 1024 query rows per core
TH = T + 2 * HW  # 1280 haloed rows
XSPLIT = 640     # xT column slab boundary


@functools.lru_cache(maxsize=1)
def _build_nc():
    import concourse.bacc as bacc
    import concourse.tile as tile
    from concourse import mybir

    f32 = mybir.dt.float32
    bf16 = mybir.dt.bfloat16
    Exp = mybir.ActivationFunctionType.Exp

    nc = bacc.Bacc("TRN2", target_bir_lowering=False, debug=False)

    xT = nc.dram_tensor("xT", [DIM, TH], bf16, kind="ExternalInput")
    wq = nc.dram_tensor("Wq", [DIM, DIM], bf16, kind="ExternalInput")
    wkv = nc.dram_tensor("Wkv", [DIM, 2 * KV * D], bf16, kind="ExternalInput")
    wo = nc.dram_tensor("Wo", [DIM, DIM], bf16, kind="ExternalInput")
    bqc = nc.dram_tensor("bqc", [128, 8], f32, kind="ExternalInput")
    # [bk (128) | bv (128) | bo (1024) | ind (1280)]
    crow = nc.dram_tensor("crow", [1, 2560], bf16, kind="ExternalInput")
    identd = nc.dram_tensor("identd", [128, 128], bf16, kind="ExternalInput")
    # [mask_lo (1024) | mask_hi (1024)] precomputed on host
    maskd = nc.dram_tensor("maskd", [128, 2048], bf16, kind="ExternalInput")
    out = nc.dram_tensor("out", [T, DIM], bf16, kind="ExternalOutput")

    with tile.TileContext(nc) as tc:
        with tc.tile_pool(name="const", bufs=1) as const, \
             tc.tile_pool(name="w", bufs=1) as wpool, \
             tc.tile_pool(name="act", bufs=1) as actp, \
             tc.tile_pool(name="attn", bufs=2) as attnp, \
             tc.tile_pool(name="ps", bufs=2, space="PSUM") as ps:

            # ---- constants come from the host: the gpsimd queue stays free
            # to issue DMAs immediately. ident first (unblocks PE warmup) ---
            ident = const.tile([128, 128], bf16, tag="ident")
            nc.gpsimd.dma_start(out=ident, in_=identd[:, :])
            # 0/1 window masks, transposed (key r, query c) orientation, for
            # the two triangular chunks; both kv-halves side by side.
            # j=0 chunk: valid where r >= c; j=2 chunk: valid where r < c.
            masks = const.tile([128, 2048], bf16, tag="masks")
            mask_lo = masks[:, 0:1024]
            mask_hi = masks[:, 1024:2048]
            ones_row = const.tile([1, 128], bf16, tag="ones")
            nc.vector.memset(ones_row, 1.0)

            # ---- PE warmup: keeps the HAM clock gate open while the input
            # DMAs stream; sized to roughly cover the arrival window --------
            junk_ps = ps.tile([128, 128], f32, tag="proj", name="junk")
            for _ in range(48):
                nc.tensor.matmul(out=junk_ps, lhsT=ident, rhs=ident,
                                 start=True, stop=True)

            # ---- DMAs: few big multi-chunk transfers, explicit queues -----
            bq_sb = const.tile([128, 8], f32, tag="bq")
            crow_sb = const.tile([1, 2560], bf16, tag="crow")
            bkr = crow_sb[:, 0:KV * D]
            bvr = crow_sb[:, KV * D:2 * KV * D]
            bor = crow_sb[:, 256:256 + DIM]
            ind_sb = crow_sb[:, 1280:1280 + TH]

            wkv_big = wpool.tile([128, 8 * 2 * KV * D], bf16, tag="wkv")
            wkv_v = wkv_big.rearrange("p (k c) -> p k c", k=8)
            wk_sb = [wkv_v[:, k, 0:KV * D] for k in range(8)]
            wv_sb = [wkv_v[:, k, KV * D:2 * KV * D] for k in range(8)]
            xT_big = wpool.tile([128, 8 * TH], bf16, tag="xT")
            xT_v = xT_big.rearrange("p (k c) -> p k c", k=8)
            xT_sb = [xT_v[:, k, :] for k in range(8)]
            wq_big = wpool.tile([128, 8 * DIM], bf16, tag="wq")
            wq_v = wq_big.rearrange("p (k c) -> p k c", k=8)
            wq_sb = [wq_v[:, k, :] for k in range(8)]
            wo_big = wpool.tile([128, 8 * DIM], bf16, tag="wo")
            wo_v = wo_big.rearrange("p (k c) -> p k c", k=8)
            wo_sb = [wo_v[:, k, :] for k in range(8)]

            xT_r = xT.rearrange("(k p) c -> p k c", k=8)
            wq_r = wq.rearrange("(k p) c -> p k c", k=8)
            wo_r = wo.rearrange("(k p) c -> p k c", k=8)
            wkv_r = wkv.rearrange("(k p) c -> p k c", k=8)

            # sync queue: first xT slab halves + wq + wo + slab1 (k 0-3)
            nc.sync.dma_start(out=xT_v[:, 0:4, 0:XSPLIT],
                              in_=xT_r[:, 0:4, 0:XSPLIT])
            nc.sync.dma_start(out=wq_v[:, 0:3, :], in_=wq_r[:, 0:3, :])
            nc.sync.dma_start(out=xT_v[:, 0:4, XSPLIT:TH],
                              in_=xT_r[:, 0:4, XSPLIT:TH])
            nc.sync.dma_start(out=wo_v[:, 0:4, :], in_=wo_r[:, 0:4, :])
            # scalar queue: the other halves
            nc.scalar.dma_start(out=xT_v[:, 4:8, 0:XSPLIT],
                                in_=xT_r[:, 4:8, 0:XSPLIT])
            nc.scalar.dma_start(out=wq_v[:, 3:6, :], in_=wq_r[:, 3:6, :])
            nc.scalar.dma_start(out=xT_v[:, 4:8, XSPLIT:TH],
                                in_=xT_r[:, 4:8, XSPLIT:TH])
            nc.scalar.dma_start(out=wo_v[:, 4:8, :], in_=wo_r[:, 4:8, :])
            # gpsimd queue (pure DMA: constants are host-precomputed)
            nc.gpsimd.dma_start(out=crow_sb, in_=crow[:, :])
            nc.gpsimd.dma_start(out=bq_sb, in_=bqc[:, :])
            nc.gpsimd.dma_start(out=wkv_v[:, :, :], in_=wkv_r)
            nc.gpsimd.dma_start(out=masks, in_=maskd[:, :])
            nc.gpsimd.dma_start(out=wq_v[:, 6:8, :], in_=wq_r[:, 6:8, :])

            # bo broadcast to all 128 partitions (K=1 outer product), used
            # by the DVE copy-out add; replaces per-tile K=1 bias matmuls.
            bo_bc = const.tile([128, DIM], bf16, tag="bo_bc")
            for n in range(2):
                bo_ps = ps.tile([128, 512], f32, tag="proj", name="bo_ps")
                nc.tensor.matmul(out=bo_ps, lhsT=ones_row,
                                 rhs=bor[:, n * 512:(n + 1) * 512],
                                 start=True, stop=True)
                nc.vector.tensor_copy(out=bo_bc[:, n * 512:(n + 1) * 512],
                                      in_=bo_ps)

            # ---- K projection over halo; zero at padded rows via ind fold -
            kT_sb = actp.tile([128, TH], bf16, tag="kT")

            def k_proj(c0, cw):
                k_ps = ps.tile([128, 512], f32, tag="proj", name="k_ps")
                for k in range(8):
                    nc.tensor.matmul(
                        out=k_ps[:, :cw], lhsT=wk_sb[k],
                        rhs=xT_sb[k][:, c0:c0 + cw],
                        start=(k == 0), stop=False)
                nc.tensor.matmul(
                    out=k_ps[:, :cw], lhsT=bkr, rhs=ind_sb[:, c0:c0 + cw],
                    start=False, stop=True)
                nc.vector.tensor_copy(out=kT_sb[:, c0:c0 + cw],
                                      in_=k_ps[:, :cw])

            # ---- V projection (keys on partitions). Layout per u-tile is
            # [V_kv0 (64) | 1 | V_kv1 (64) | 1]: the ones column appended to
            # each kv-slice makes the probs@[V|1] matmul emit the softmax
            # denominator as output column 64 for free. ---------------------
            NU = TH // 128
            v_sb = actp.tile([128, NU * 130], bf16, tag="V")
            v_view = v_sb.rearrange("p (u g c) -> p u g c", u=NU, g=2)
            nc.vector.memset(v_view[:, :, :, 64:65], 1.0)

            def v_proj(ut):
                v_ps = ps.tile([128, 512], f32, tag="proj", name="v_ps")
                for k in range(8):
                    nc.tensor.matmul(
                        out=v_ps[:, :KV * D],
                        lhsT=xT_sb[k][:, ut * 128:(ut + 1) * 128],
                        rhs=wv_sb[k], start=(k == 0), stop=False)
                nc.tensor.matmul(
                    out=v_ps[:, :KV * D],
                    lhsT=ind_sb[:, ut * 128:(ut + 1) * 128], rhs=bvr,
                    start=False, stop=True)
                nc.vector.tensor_copy(
                    out=v_view[:, ut, :, 0:64],
                    in_=v_ps[:, :KV * D].rearrange("p (g c) -> p g c", g=2))

            # ---- Q projection: qT tile g holds heads (2g, 2g+1) along the
            # free dim and heads (+8) on the upper partition half ------------
            qT_sb = []
            for g in range(2):
                t_qt = actp.tile([128, 4 * T], bf16, tag=f"qT{g}",
                                 name=f"qT{g}")
                qT_sb.append(t_qt)

            def q_proj(m, n):
                q_ps = ps.tile([128, 512], f32, tag="proj", name="q_ps")
                for k in range(8):
                    nc.tensor.matmul(
                        out=q_ps,
                        lhsT=wq_sb[k][:, m * 128:(m + 1) * 128],
                        rhs=xT_sb[k][:, HW + n * 512: HW + (n + 1) * 512],
                        start=(k == 0), stop=(k == 7))
                off = (m % 4) * T + n * 512
                nc.vector.tensor_scalar_add(
                    out=qT_sb[m // 4][:, off:off + 512], in0=q_ps,
                    scalar1=bq_sb[:, m:m + 1])

            # ---- pre-attention work (data-arrival ordered) -----------------
            k_proj(0, 512)
            k_proj(512, XSPLIT - 512)
            for ut in range(3):
                v_proj(ut)
            for m in range(8):
                q_proj(m, 0)

            # ---- attention + output transpose + (skewed) out-projection ---
            attnT = actp.tile([128, 8 * T], bf16, tag="attnT")
            attnT_v = attnT.rearrange("p (k t) -> p k t", k=8)

            def out_proj(mt):
                out_t = attnp.tile([128, DIM], bf16, tag="outt")
                for n in range(2):
                    o2 = ps.tile([128, 512], f32, tag="proj", name="o2_ps")
                    for k in range(8):
                        nc.tensor.matmul(
                            out=o2,
                            lhsT=attnT[:, k * T + mt * 128:
                                       k * T + (mt + 1) * 128],
                            rhs=wo_sb[k][:, n * 512:(n + 1) * 512],
                            start=(k == 0), stop=(k == 7))
                    nc.vector.tensor_add(
                        out=out_t[:, n * 512:(n + 1) * 512], in0=o2,
                        in1=bo_bc[:, n * 512:(n + 1) * 512])
                    nc.sync.dma_start(
                        out=out[mt * 128:(mt + 1) * 128,
                                n * 512:(n + 1) * 512],
                        in_=out_t[:, n * 512:(n + 1) * 512])

            for mt in range(8):
                # skewed out-projection: early tiles run it after their
                # attention (Wo may still be in flight); late tiles run it
                # first so only out_proj(7) remains after the last attention.
                if mt >= 4:
                    out_proj(mt - 1)
                qcol = mt * 128
                u0 = qcol  # halo col of first attended key
                attn_t = attnp.tile([128, DIM], bf16, tag="attn")
                for gg in range(2):
                    qv = qT_sb[gg].rearrange("p (i t) -> p i t", i=4)
                    p2s = []
                    for j in range(3):
                        # both kv-halves of key-chunk j, row-packed into one
                        # 2-bank PSUM tile; four same-kv heads stream as one
                        # N=512 rhs per half.
                        s2 = ps.tile([128, 1024], f32, tag="s2", bufs=2,
                                     name="s2")
                        for half in range(2):
                            nc.tensor.matmul(
                                out=s2[:, half * 512:(half + 1) * 512],
                                lhsT=kT_sb[half * 64:(half + 1) * 64,
                                           u0 + j * 128:u0 + (j + 1) * 128],
                                rhs=qv[half * 64:(half + 1) * 64, :,
                                       qcol:qcol + 128],
                                start=True, stop=True,
                                tile_position=(64 * half, 0))
                        p2 = attnp.tile([128, 1024], bf16, tag="P", bufs=6,
                                        name="p2")
                        nc.scalar.activation(out=p2, in_=s2, func=Exp)
                        if j == 0:
                            nc.vector.tensor_mul(p2, p2, mask_lo)
                        elif j == 2:
                            nc.vector.tensor_mul(p2, p2, mask_hi)
                        p2s.append(p2)
                    for half in range(2):
                        # 4 heads share one PSUM bank: [a, 0:64]=attn out,
                        # [a, 64]=softmax denominator.
                        o4 = ps.tile([128, 260], f32, tag="o4", bufs=2,
                                     name="o4")
                        o4v = o4.rearrange("p (a c) -> p a c", a=4)
                        for a in range(4):
                            for j in range(3):
                                nc.tensor.matmul(
                                    out=o4v[:, a, :],
                                    lhsT=p2s[j][:, half * 512 + a * 128:
                                                half * 512 + (a + 1) * 128],
                                    rhs=v_view[:, mt + j, half, 0:65],
                                    start=(j == 0), stop=(j == 2))
                        rc4 = attnp.tile([128, 4], f32, tag="rc4", bufs=4,
                                         name="rc4")
                        nc.vector.reciprocal(out=rc4[:, :].unsqueeze(2),
                                             in_=o4v[:, :, 64:65])
                        hbase = (4 * gg + 8 * half) * 64
                        dst = attn_t[:, hbase:hbase + 256].rearrange(
                            "p (a d) -> p a d", a=4)
                        nc.vector.tensor_mul(
                            dst, o4v[:, :, 0:64],
                            rc4[:, :].unsqueeze(2).broadcast_to([128, 4, 64]))
                # transpose attn rows (t) x cols (hd) -> attnT k-tiles
                for g in range(3):
                    kcnt = 3 if g < 2 else 2
                    at_ps = ps.tile([128, 384], bf16, tag="proj", bufs=2,
                                    name="at_ps")
                    for jj in range(kcnt):
                        kk = g * 3 + jj
                        nc.tensor.matmul(
                            out=at_ps[:, jj * 128:(jj + 1) * 128],
                            lhsT=attn_t[:, kk * 128:(kk + 1) * 128],
                            rhs=ident, is_transpose=True,
                            start=(jj == 0), stop=(jj == kcnt - 1))
                    src = at_ps[:, :kcnt * 128].rearrange(
                        "p (j c) -> p j c", j=kcnt)
                    dst = attnT_v[:, g * 3:g * 3 + kcnt, qcol:qcol + 128]
                    nc.vector.tensor_copy(out=dst, in_=src)

                if 1 <= mt <= 3:
                    out_proj(mt - 1)
                # just-in-time projection work keyed to DMA arrival order
                if mt == 0:
                    v_proj(3)
                    v_proj(4)
                    k_proj(XSPLIT, 512)
                elif mt == 1:
                    k_proj(XSPLIT + 512, TH - XSPLIT - 512)
                    v_proj(5)
                elif mt == 2:
                    v_proj(6)
                    for m in range(4):
                        q_proj(m, 1)
                elif mt == 3:
                    v_proj(7)
                    for m in range(4, 8):
                        q_proj(m, 1)
                elif mt == 4:
                    v_proj(8)
                    v_proj(9)
            out_proj(7)

    nc.compile()
    return nc


def _host_prep(x, Wq, bq, Wk, bk, Wv, bv, Wo, bo):
    import ml_dtypes
    bf16 = ml_dtypes.bfloat16

    # permute Wq/bq columns so qT m-tile holds head m on partitions 0-63 and
    # head m+8 on partitions 64-127 (enables row-packed score matmuls)
    idx = np.empty(DIM, dtype=np.int64)
    for m in range(8):
        for j in range(128):
            h = m if j < 64 else m + 8
            idx[m * 128 + j] = h * D + (j % 64)
    wq_p = np.ascontiguousarray(Wq[:, idx]).astype(bf16)
    bq_p = bq[idx].astype(np.float32).reshape(8, 128).T.copy()  # (128, 8)
    wkv_b = np.ascontiguousarray(
        np.concatenate([Wk, Wv], axis=1)).astype(bf16)
    wo_b = np.ascontiguousarray(Wo).astype(bf16)

    ident_h = np.eye(128, dtype=np.float32).astype(bf16)
    r, c = np.arange(128)[:, None], np.arange(128)[None, :]
    mask_h = np.concatenate(
        [np.tile((r >= c).astype(np.float32), (1, 8)),
         np.tile((r < c).astype(np.float32), (1, 8))], axis=1).astype(bf16)

    in_maps = []
    for c in range(NCORES):
        b, qt = c // QT, c % QT
        lo, hi = qt * T - HW, qt * T + T + HW
        xs = np.zeros((TH, DIM), dtype=np.float32)
        s0, s1 = max(lo, 0), min(hi, S)
        xs[s0 - lo:s1 - lo] = x[b, s0:s1]
        crow = np.zeros((1, 2560), dtype=np.float32)
        crow[0, 0:128] = bk
        crow[0, 128:256] = bv
        crow[0, 256:1280] = bo
        crow[0, 1280 + (s0 - lo):1280 + (s1 - lo)] = 1.0
        in_maps.append({
            "xT": np.ascontiguousarray(xs.T).astype(bf16),
            "Wq": wq_p, "Wkv": wkv_b, "Wo": wo_b,
            "bqc": bq_p, "crow": crow.astype(bf16),
            "identd": ident_h, "maskd": mask_h,
        })
    return in_maps


def kernel(x, Wq, bq, Wk, bk, Wv, bv, Wo, bo):
    from concourse.bass_utils import run_bass_kernel_spmd

    x, Wq, bq, Wk, bk, Wv, bv, Wo, bo = (
        np.asarray(a, dtype=np.float32)
        for a in (x, Wq, bq, Wk, bk, Wv, bv, Wo, bo))
    nc = _build_nc()
    in_maps = _host_prep(x, Wq, bq, Wk, bk, Wv, bv, Wo, bo)
    res = run_bass_kernel_spmd(nc, in_maps, core_ids=list(range(NCORES)))
    out = np.empty((B, S, DIM), dtype=np.float32)
    for c in range(NCORES):
        b, qt = c // QT, c % QT
        out[b, qt * T:(qt + 1) * T] = res.results[c]["out"].astype(np.float32)
    return out


# revision 18
# speedup vs baseline: 1.0744x; 1.0126x over previous
"""Trainium2 Bass kernel for sliding-window GQA attention block.

Reference computation (B=2, S=4096, DIM=1024, H=16 q-heads, KV=2 kv-heads,
D=64, W=256 window):
    q = x@Wq + bq ; k = x@Wk + bk ; v = x@Wv + bv        (GQA repeat kv x8)
    local attention: query t attends keys [t-128, t+128) (zero-padded edges,
    no 1/sqrt(d) scaling), softmax, out = probs@v
    y = out@Wo + bo

Sharding: 8 cores = batch(2) x seq-quarter(4). Each core computes 1024
query rows end-to-end (all 16 heads) from a 1280-row haloed x slice.
No cross-core communication; host pads/transposes/gathers.

On-device pipeline per core (all matmuls bf16, fp32 PSUM accumulation):
  QKV projections (QK biases folded via per-partition DVE add / K=1
  indicator-row matmuls, which also zero K,V at padded halo rows) ->
  scores computed directly TRANSPOSED (S^T[u,t], keys on partitions;
  both kv-halves of one key-chunk row-packed into a single 2-bank PSUM
  tile so one exp covers them) -> exp on ScalarE (the only ScalarE op;
  all copies live on the DVE) -> 0/1 band-mask multiply on VectorE for
  the two triangular chunks only -> probs @ [V|1] grouped 4 heads per
  PSUM bank (the ones column emits the softmax denominator at column
  64 of each head's 65-col group) -> one strided reciprocal + one
  broadcast tensor_tensor multiply normalizes 4 heads at once ->
  PE-transpose of the 128x128 attn blocks -> out-projection with the
  bias folded via a precomputed broadcast row added during the DVE
  PSUM->SBUF copy. Input DMAs are column-sliced and ordered so the
  K/V-projection data lands first, spread over 4 engine queues, while
  junk identity matmuls warm the PE HAM clock gate during the wait.
"""

import functools
import numpy as np

B, S, DIM = 2, 4096, 1024
H, KV, D = 16, 2, 64
W, HW = 256, 128
NCORES = 8
QT = 4           # sequence quarters
T = S // QT      # 1024 query rows per core
TH = T + 2 * HW  # 1280 haloed rows
XSPLIT = 640     # xT column slab boundary


@functools.lru_cache(maxsize=1)
def _build_nc():
    import concourse.bacc as bacc
    import concourse.tile as tile
    from concourse import mybir

    f32 = mybir.dt.float32
    bf16 = mybir.dt.bfloat16
    Exp = mybir.ActivationFunctionType.Exp

    nc = bacc.Bacc("TRN2", target_bir_lowering=False, debug=False)

    xT = nc.dram_tensor("xT", [DIM, TH], bf16, kind="ExternalInput")
    wq = nc.dram_tensor("Wq", [DIM, DIM], bf16, kind="ExternalInput")
    wkv = nc.dram_tensor("Wkv", [DIM, 2 * KV * D], bf16, kind="ExternalInput")
    wo = nc.dram_tensor("Wo", [DIM, DIM], bf16, kind="ExternalInput")
    bqc = nc.dram_tensor("bqc", [128, 8], f32, kind="ExternalInput")
    # [bk (128) | bv (128) | bo (1024) | ind (1280)]
    crow = nc.dram_tensor("crow", [1, 2560], bf16, kind="ExternalInput")
    identd = nc.dram_tensor("identd", [128, 128], bf16, kind="ExternalInput")
    out = nc.dram_tensor("out", [T, DIM], bf16, kind="ExternalOutput")

    with tile.TileContext(nc) as tc:
        with tc.tile_pool(name="const", bufs=1) as const, \
             tc.tile_pool(name="w", bufs=1) as wpool, \
             tc.tile_pool(name="act", bufs=1) as actp, \
             tc.tile_pool(name="attn", bufs=2) as attnp, \
             tc.tile_pool(name="ps", bufs=2, space="PSUM") as ps:

            ones_row = const.tile([1, 128], bf16, tag="ones")
            nc.vector.memset(ones_row, 1.0)

            # ---- PE warmup: keeps the HAM clock gate open while the input
            # DMAs stream. Uses a locally-memset zero tile so the warmup has
            # no DMA dependency and starts right after the preamble. --------
            junk_rhs = const.tile([128, 512], bf16, tag="junk_rhs")
            nc.vector.memset(junk_rhs, 0.0)
            junk_ps = ps.tile([128, 512], f32, tag="proj", name="junk")
            for _ in range(32):
                nc.tensor.matmul(out=junk_ps, lhsT=junk_rhs[:, 0:128],
                                 rhs=junk_rhs, start=True, stop=True)

            # ---- DMAs: few big multi-chunk transfers, explicit queues -----
            bq_sb = const.tile([128, 8], f32, tag="bq")
            crow_sb = const.tile([1, 2560], bf16, tag="crow")
            bkr = crow_sb[:, 0:KV * D]
            bvr = crow_sb[:, KV * D:2 * KV * D]
            bor = crow_sb[:, 256:256 + DIM]
            ind_sb = crow_sb[:, 1280:1280 + TH]

            wkv_big = wpool.tile([128, 8 * 2 * KV * D], bf16, tag="wkv")
            wkv_v = wkv_big.rearrange("p (k c) -> p k c", k=8)
            wk_sb = [wkv_v[:, k, 0:KV * D] for k in range(8)]
            wv_sb = [wkv_v[:, k, KV * D:2 * KV * D] for k in range(8)]
            xT_big = wpool.tile([128, 8 * TH], bf16, tag="xT")
            xT_v = xT_big.rearrange("p (k c) -> p k c", k=8)
            xT_sb = [xT_v[:, k, :] for k in range(8)]
            wq_big = wpool.tile([128, 8 * DIM], bf16, tag="wq")
            wq_v = wq_big.rearrange("p (k c) -> p k c", k=8)
            wq_sb = [wq_v[:, k, :] for k in range(8)]
            wo_big = wpool.tile([128, 8 * DIM], bf16, tag="wo")
            wo_v = wo_big.rearrange("p (k c) -> p k c", k=8)
            wo_sb = [wo_v[:, k, :] for k in range(8)]

            xT_r = xT.rearrange("(k p) c -> p k c", k=8)
            wq_r = wq.rearrange("(k p) c -> p k c", k=8)
            wo_r = wo.rearrange("(k p) c -> p k c", k=8)
            wkv_r = wkv.rearrange("(k p) c -> p k c", k=8)

            # Wq arrives as per-m column slices so each qT tile's projection
            # can start as soon as its own 256KB lands (overlaps the DMA).
            def wq_m(eng, m):
                eng.dma_start(out=wq_v[:, :, m * 128:(m + 1) * 128],
                              in_=wq_r[:, :, m * 128:(m + 1) * 128])

            # sync queue: xT slab halves + wq m-slices + slab1 + wo (k 0-3)
            nc.sync.dma_start(out=xT_v[:, 0:4, 0:XSPLIT],
                              in_=xT_r[:, 0:4, 0:XSPLIT])
            wq_m(nc.sync, 0)
            wq_m(nc.sync, 2)
            wq_m(nc.sync, 4)
            wq_m(nc.sync, 6)
            nc.sync.dma_start(out=xT_v[:, 0:4, XSPLIT:TH],
                              in_=xT_r[:, 0:4, XSPLIT:TH])
            nc.sync.dma_start(out=wo_v[:, 0:4, :], in_=wo_r[:, 0:4, :])
            # scalar queue: the other halves
            nc.scalar.dma_start(out=xT_v[:, 4:8, 0:XSPLIT],
                                in_=xT_r[:, 4:8, 0:XSPLIT])
            wq_m(nc.scalar, 1)
            wq_m(nc.scalar, 3)
            wq_m(nc.scalar, 5)
            wq_m(nc.scalar, 7)
            nc.scalar.dma_start(out=xT_v[:, 4:8, XSPLIT:TH],
                                in_=xT_r[:, 4:8, XSPLIT:TH])
            nc.scalar.dma_start(out=wo_v[:, 4:8, :], in_=wo_r[:, 4:8, :])
            # gpsimd queue: small constants + wkv, then the mask build below
            ident = const.tile([128, 128], bf16, tag="ident")
            nc.gpsimd.dma_start(out=ident, in_=identd[:, :])
            nc.gpsimd.dma_start(out=crow_sb, in_=crow[:, :])
            nc.gpsimd.dma_start(out=bq_sb, in_=bqc[:, :])
            nc.gpsimd.dma_start(out=wkv_v[:, :, :], in_=wkv_r)

            # 0/1 window masks, transposed (key r, query c) orientation, for
            # the two triangular chunks; both kv-halves side by side.
            # j=0 chunk: valid where r >= c; j=2 chunk: valid where r < c.
            # Built on gpsimd AFTER its DMA issues (the engine is idle then).
            masks = const.tile([128, 2048], bf16, tag="masks")
            mask_lo = masks[:, 0:1024]
            mask_hi = masks[:, 1024:2048]
            nc.gpsimd.memset(mask_lo, 1.0)
            nc.gpsimd.memset(mask_hi, 1.0)
            for blk2 in range(0, 1024, 128):
                nc.gpsimd.affine_select(
                    out=mask_lo[:, blk2:blk2 + 128],
                    in_=mask_lo[:, blk2:blk2 + 128],
                    compare_op=mybir.AluOpType.is_ge,
                    fill=0.0, base=0, pattern=[[-1, 128]],
                    channel_multiplier=1)
                nc.gpsimd.affine_select(
                    out=mask_hi[:, blk2:blk2 + 128],
                    in_=mask_hi[:, blk2:blk2 + 128],
                    compare_op=mybir.AluOpType.is_ge,
                    fill=0.0, base=-1, pattern=[[1, 128]],
                    channel_multiplier=-1)

            # bo broadcast to all 128 partitions (K=1 outer product), used
            # by the DVE copy-out add; replaces per-tile K=1 bias matmuls.
            bo_bc = const.tile([128, DIM], bf16, tag="bo_bc")
            for n in range(2):
                bo_ps = ps.tile([128, 512], f32, tag="proj", name="bo_ps")
                nc.tensor.matmul(out=bo_ps, lhsT=ones_row,
                                 rhs=bor[:, n * 512:(n + 1) * 512],
                                 start=True, stop=True)
                nc.vector.tensor_copy(out=bo_bc[:, n * 512:(n + 1) * 512],
                                      in_=bo_ps)

            # ---- K projection over halo; zero at padded rows via ind fold -
            kT_sb = actp.tile([128, TH], bf16, tag="kT")

            def k_proj(c0, cw):
                k_ps = ps.tile([128, 512], f32, tag="proj", name="k_ps")
                for k in range(8):
                    nc.tensor.matmul(
                        out=k_ps[:, :cw], lhsT=wk_sb[k],
                        rhs=xT_sb[k][:, c0:c0 + cw],
                        start=(k == 0), stop=False)
                nc.tensor.matmul(
                    out=k_ps[:, :cw], lhsT=bkr, rhs=ind_sb[:, c0:c0 + cw],
                    start=False, stop=True)
                nc.vector.tensor_copy(out=kT_sb[:, c0:c0 + cw],
                                      in_=k_ps[:, :cw])

            # ---- V projection (keys on partitions). Layout per u-tile is
            # [V_kv0 (64) | 1 | V_kv1 (64) | 1]: the ones column appended to
            # each kv-slice makes the probs@[V|1] matmul emit the softmax
            # denominator as output column 64 for free. ---------------------
            NU = TH // 128
            v_sb = actp.tile([128, NU * 130], bf16, tag="V")
            v_view = v_sb.rearrange("p (u g c) -> p u g c", u=NU, g=2)
            nc.vector.memset(v_view[:, :, :, 64:65], 1.0)

            def v_proj(ut):
                v_ps = ps.tile([128, 512], f32, tag="proj", name="v_ps")
                for k in range(8):
                    nc.tensor.matmul(
                        out=v_ps[:, :KV * D],
                        lhsT=xT_sb[k][:, ut * 128:(ut + 1) * 128],
                        rhs=wv_sb[k], start=(k == 0), stop=False)
                nc.tensor.matmul(
                    out=v_ps[:, :KV * D],
                    lhsT=ind_sb[:, ut * 128:(ut + 1) * 128], rhs=bvr,
                    start=False, stop=True)
                nc.vector.tensor_copy(
                    out=v_view[:, ut, :, 0:64],
                    in_=v_ps[:, :KV * D].rearrange("p (g c) -> p g c", g=2))

            # ---- Q projection: qT tile g holds heads (2g, 2g+1) along the
            # free dim and heads (+8) on the upper partition half ------------
            qT_sb = []
            for g in range(2):
                t_qt = actp.tile([128, 4 * T], bf16, tag=f"qT{g}",
                                 name=f"qT{g}")
                qT_sb.append(t_qt)

            def q_proj(m, n):
                q_ps = ps.tile([128, 512], f32, tag="proj", name="q_ps")
                for k in range(8):
                    nc.tensor.matmul(
                        out=q_ps,
                        lhsT=wq_sb[k][:, m * 128:(m + 1) * 128],
                        rhs=xT_sb[k][:, HW + n * 512: HW + (n + 1) * 512],
                        start=(k == 0), stop=(k == 7))
                off = (m % 4) * T + n * 512
                nc.vector.tensor_scalar_add(
                    out=qT_sb[m // 4][:, off:off + 512], in0=q_ps,
                    scalar1=bq_sb[:, m:m + 1])

            # ---- pre-attention work (data-arrival ordered) -----------------
            k_proj(0, 512)
            k_proj(512, XSPLIT - 512)
            for ut in range(3):
                v_proj(ut)
            for m in range(8):
                q_proj(m, 0)

            # ---- attention + output transpose + (skewed) out-projection ---
            attnT = actp.tile([128, 8 * T], bf16, tag="attnT")
            attnT_v = attnT.rearrange("p (k t) -> p k t", k=8)

            def out_proj_half(mt, n):
                out_t = attnp.tile([128, 512], bf16, tag="outt")
                o2 = ps.tile([128, 512], f32, tag="proj", name="o2_ps")
                for k in range(8):
                    nc.tensor.matmul(
                        out=o2,
                        lhsT=attnT[:, k * T + mt * 128:
                                   k * T + (mt + 1) * 128],
                        rhs=wo_sb[k][:, n * 512:(n + 1) * 512],
                        start=(k == 0), stop=(k == 7))
                nc.vector.tensor_add(out=out_t, in0=o2,
                                     in1=bo_bc[:, n * 512:(n + 1) * 512])
                nc.sync.dma_start(
                    out=out[mt * 128:(mt + 1) * 128, n * 512:(n + 1) * 512],
                    in_=out_t)

            def out_proj(mt):
                out_proj_half(mt, 0)
                out_proj_half(mt, 1)

            # PE filler work per (tile, slot): slots 0/1 run between the two
            # attention head-groups (covering the exp latency with
            # independent matmuls), slot 2 after the transposes.
            fillers = {
                0: ([lambda: v_proj(3)], [lambda: v_proj(4)],
                    [lambda: k_proj(XSPLIT, 512)]),
                1: ([lambda: k_proj(XSPLIT + 512, TH - XSPLIT - 512)],
                    [lambda: v_proj(5)], [lambda: out_proj(0)]),
                2: ([lambda: q_proj(0, 1), lambda: q_proj(1, 1)],
                    [lambda: v_proj(6), lambda: q_proj(2, 1)],
                    [lambda: q_proj(3, 1), lambda: out_proj(1)]),
                3: ([lambda: q_proj(4, 1), lambda: q_proj(5, 1)],
                    [lambda: v_proj(7), lambda: q_proj(6, 1)],
                    [lambda: q_proj(7, 1), lambda: out_proj(2)]),
                4: ([lambda: out_proj_half(3, 0), lambda: v_proj(8)],
                    [lambda: out_proj_half(3, 1), lambda: v_proj(9)], []),
                5: ([lambda: out_proj_half(4, 0)],
                    [lambda: out_proj_half(4, 1)], []),
                6: ([lambda: out_proj_half(5, 0)],
                    [lambda: out_proj_half(5, 1)], []),
                7: ([lambda: out_proj_half(6, 0)],
                    [lambda: out_proj_half(6, 1)], []),
            }

            for mt in range(8):
                qcol = mt * 128
                u0 = qcol  # halo col of first attended key
                attn_t = attnp.tile([128, DIM], bf16, tag="attn")
                for gg in range(2):
                    qv = qT_sb[gg].rearrange("p (i t) -> p i t", i=4)
                    p2s = []
                    for j in range(3):
                        # both kv-halves of key-chunk j, row-packed into one
                        # 2-bank PSUM tile; four same-kv heads stream as one
                        # N=512 rhs per half.
                        s2 = ps.tile([128, 1024], f32, tag="s2", bufs=2,
                                     name="s2")
                        for half in range(2):
                            nc.tensor.matmul(
                                out=s2[:, half * 512:(half + 1) * 512],
                                lhsT=kT_sb[half * 64:(half + 1) * 64,
                                           u0 + j * 128:u0 + (j + 1) * 128],
                                rhs=qv[half * 64:(half + 1) * 64, :,
                                       qcol:qcol + 128],
                                start=True, stop=True,
                                tile_position=(64 * half, 0))
                        p2 = attnp.tile([128, 1024], bf16, tag="P", bufs=6,
                                        name="p2")
                        nc.scalar.activation(out=p2, in_=s2, func=Exp)
                        if j == 0:
                            nc.vector.tensor_mul(p2, p2, mask_lo)
                        elif j == 2:
                            nc.vector.tensor_mul(p2, p2, mask_hi)
                        p2s.append(p2)
                    for half in range(2):
                        # 4 heads share one PSUM bank: [a, 0:64]=attn out,
                        # [a, 64]=softmax denominator.
                        o4 = ps.tile([128, 260], f32, tag="o4", bufs=2,
                                     name="o4")
                        o4v = o4.rearrange("p (a c) -> p a c", a=4)
                        for a in range(4):
                            for j in range(3):
                                nc.tensor.matmul(
                                    out=o4v[:, a, :],
                                    lhsT=p2s[j][:, half * 512 + a * 128:
                                                half * 512 + (a + 1) * 128],
                                    rhs=v_view[:, mt + j, half, 0:65],
                                    start=(j == 0), stop=(j == 2))
                        rc4 = attnp.tile([128, 4], f32, tag="rc4", bufs=4,
                                         name="rc4")
                        nc.vector.reciprocal(out=rc4[:, :].unsqueeze(2),
                                             in_=o4v[:, :, 64:65])
                        hbase = (4 * gg + 8 * half) * 64
                        dst = attn_t[:, hbase:hbase + 256].rearrange(
                            "p (a d) -> p a d", a=4)
                        nc.vector.tensor_mul(
                            dst, o4v[:, :, 0:64],
                            rc4[:, :].unsqueeze(2).broadcast_to([128, 4, 64]))
                    for f in fillers[mt][gg]:
                        f()
                # transpose attn rows (t) x cols (hd) -> attnT k-tiles
                for g in range(3):
                    kcnt = 3 if g < 2 else 2
                    at_ps = ps.tile([128, 384], bf16, tag="proj", bufs=2,
                                    name="at_ps")
                    for jj in range(kcnt):
                        kk = g * 3 + jj
                        nc.tensor.matmul(
                            out=at_ps[:, jj * 128:(jj + 1) * 128],
                            lhsT=attn_t[:, kk * 128:(kk + 1) * 128],
                            rhs=ident, is_transpose=True,
                            start=(jj == 0), stop=(jj == kcnt - 1))
                    src = at_ps[:, :kcnt * 128].rearrange(
                        "p (j c) -> p j c", j=kcnt)
                    dst = attnT_v[:, g * 3:g * 3 + kcnt, qcol:qcol + 128]
                    if mt >= 6:
                        # ScalarE is idle at the kernel tail; keep the DVE
                        # off the critical path into the last out-projections
                        nc.scalar.copy(out=dst, in_=src)
                    else:
                        nc.vector.tensor_copy(out=dst, in_=src)
                for f in fillers[mt][2]:
                    f()
            out_proj(7)

    nc.compile()
    return nc


def _host_prep(x, Wq, bq, Wk, bk, Wv, bv, Wo, bo):
    import ml_dtypes
    bf16 = ml_dtypes.bfloat16

    # permute Wq/bq columns so qT m-tile holds head m on partitions 0-63 and
    # head m+8 on partitions 64-127 (enables row-packed score matmuls)
    idx = np.empty(DIM, dtype=np.int64)
    for m in range(8):
        for j in range(128):
            h = m if j < 64 else m + 8
            idx[m * 128 + j] = h * D + (j % 64)
    wq_p = np.ascontiguousarray(Wq[:, idx]).astype(bf16)
    bq_p = bq[idx].astype(np.float32).reshape(8, 128).T.copy()  # (128, 8)
    wkv_b = np.ascontiguousarray(
        np.concatenate([Wk, Wv], axis=1)).astype(bf16)
    wo_b = np.ascontiguousarray(Wo).astype(bf16)

    ident_h = np.eye(128, dtype=np.float32).astype(bf16)
    r, c = np.arange(128)[:, None], np.arange(128)[None, :]
    mask_h = np.concatenate(
        [np.tile((r >= c).astype(np.float32), (1, 8)),
         np.tile((r < c).astype(np.float32), (1, 8))], axis=1).astype(bf16)

    in_maps = []
    for c in range(NCORES):
        b, qt = c // QT, c % QT
        lo, hi = qt * T - HW, qt * T + T + HW
        xs = np.zeros((TH, DIM), dtype=np.float32)
        s0, s1 = max(lo, 0), min(hi, S)
        xs[s0 - lo:s1 - lo] = x[b, s0:s1]
        crow = np.zeros((1, 2560), dtype=np.float32)
        crow[0, 0:128] = bk
        crow[0, 128:256] = bv
        crow[0, 256:1280] = bo
        crow[0, 1280 + (s0 - lo):1280 + (s1 - lo)] = 1.0
        in_maps.append({
            "xT": np.ascontiguousarray(xs.T).astype(bf16),
            "Wq": wq_p, "Wkv": wkv_b, "Wo": wo_b,
            "bqc": bq_p, "crow": crow.astype(bf16),
            "identd": ident_h, "maskd": mask_h,
        })
    return in_maps


def kernel(x, Wq, bq, Wk, bk, Wv, bv, Wo, bo):
    from concourse.bass_utils import run_bass_kernel_spmd

    x, Wq, bq, Wk, bk, Wv, bv, Wo, bo = (
        np.asarray(a, dtype=np.float32)
        for a in (x, Wq, bq, Wk, bk, Wv, bv, Wo, bo))
    nc = _build_nc()
    in_maps = _host_prep(x, Wq, bq, Wk, bk, Wv, bv, Wo, bo)
    res = run_bass_kernel_spmd(nc, in_maps, core_ids=list(range(NCORES)))
    out = np.empty((B, S, DIM), dtype=np.float32)
    for c in range(NCORES):
        b, qt = c // QT, c % QT
        out[b, qt * T:(qt + 1) * T] = res.results[c]["out"].astype(np.float32)
    return out


# revision 22
# speedup vs baseline: 1.0866x; 1.0114x over previous
"""Trainium2 Bass kernel for sliding-window GQA attention block.

Reference computation (B=2, S=4096, DIM=1024, H=16 q-heads, KV=2 kv-heads,
D=64, W=256 window):
    q = x@Wq + bq ; k = x@Wk + bk ; v = x@Wv + bv        (GQA repeat kv x8)
    local attention: query t attends keys [t-128, t+128) (zero-padded edges,
    no 1/sqrt(d) scaling), softmax, out = probs@v
    y = out@Wo + bo

Sharding: 8 cores = batch(2) x seq-quarter(4). Each core computes 1024
query rows end-to-end (all 16 heads) from a 1280-row haloed x slice.
No cross-core communication; host pads/transposes/gathers.

On-device pipeline per core (all matmuls bf16, fp32 PSUM accumulation):
  QKV projections (QK biases folded via per-partition DVE add / K=1
  indicator-row matmuls, which also zero K,V at padded halo rows) ->
  scores computed directly TRANSPOSED (S^T[u,t], keys on partitions;
  both kv-halves of one key-chunk row-packed into a single 2-bank PSUM
  tile so one exp covers them) -> exp on ScalarE (the only ScalarE op;
  all copies live on the DVE) -> 0/1 band-mask multiply on VectorE for
  the two triangular chunks only -> probs @ [V|1] grouped 4 heads per
  PSUM bank (the ones column emits the softmax denominator at column
  64 of each head's 65-col group) -> one strided reciprocal + one
  broadcast tensor_tensor multiply normalizes 4 heads at once ->
  PE-transpose of the 128x128 attn blocks -> out-projection with the
  bias folded via a precomputed broadcast row added during the DVE
  PSUM->SBUF copy. Input DMAs are column-sliced and ordered so the
  K/V-projection data lands first, spread over 4 engine queues, while
  junk identity matmuls warm the PE HAM clock gate during the wait.
"""

import functools
import numpy as np

B, S, DIM = 2, 4096, 1024
H, KV, D = 16, 2, 64
W, HW = 256, 128
NCORES = 8
QT = 4           # sequence quarters
T = S // QT      # 1024 query rows per core
TH = T + 2 * HW  # 1280 haloed rows
XSPLIT = 640     # xT column slab boundary


@functools.lru_cache(maxsize=1)
def _build_nc():
    import concourse.bacc as bacc
    import concourse.tile as tile
    from concourse import mybir

    f32 = mybir.dt.float32
    bf16 = mybir.dt.bfloat16
    Exp = mybir.ActivationFunctionType.Exp

    nc = bacc.Bacc("TRN2", target_bir_lowering=False, debug=False)

    xT = nc.dram_tensor("xT", [DIM, TH], bf16, kind="ExternalInput")
    wq = nc.dram_tensor("Wq", [DIM, DIM], bf16, kind="ExternalInput")
    wkv = nc.dram_tensor("Wkv", [DIM, 2 * KV * D], bf16, kind="ExternalInput")
    wo = nc.dram_tensor("Wo", [DIM, DIM], bf16, kind="ExternalInput")
    bqc = nc.dram_tensor("bqc", [128, 8], f32, kind="ExternalInput")
    # [bk (128) | bv (128) | bo (1024) | ind (1280)]
    crow = nc.dram_tensor("crow", [1, 2560], bf16, kind="ExternalInput")
    identd = nc.dram_tensor("identd", [128, 128], bf16, kind="ExternalInput")
    out = nc.dram_tensor("out", [T, DIM], bf16, kind="ExternalOutput")

    with tile.TileContext(nc) as tc:
        with tc.tile_pool(name="const", bufs=1) as const, \
             tc.tile_pool(name="w", bufs=1) as wpool, \
             tc.tile_pool(name="act", bufs=1) as actp, \
             tc.tile_pool(name="attn", bufs=2) as attnp, \
             tc.tile_pool(name="ps", bufs=2, space="PSUM") as ps:

            ones_row = const.tile([1, 128], bf16, tag="ones")
            nc.vector.memset(ones_row, 1.0)

            # ---- PE warmup: keeps the HAM clock gate open while the input
            # DMAs stream. Uses a locally-memset zero tile so the warmup has
            # no DMA dependency and starts right after the preamble. --------
            junk_rhs = const.tile([128, 512], bf16, tag="junk_rhs")
            nc.vector.memset(junk_rhs, 0.0)
            junk_ps = ps.tile([128, 512], f32, tag="proj", name="junk")
            for _ in range(28):
                nc.tensor.matmul(out=junk_ps, lhsT=junk_rhs[:, 0:128],
                                 rhs=junk_rhs, start=True, stop=True)

            # ---- DMAs: few big multi-chunk transfers, explicit queues -----
            bq_sb = const.tile([128, 8], f32, tag="bq")
            crow_sb = const.tile([1, 2560], bf16, tag="crow")
            bkr = crow_sb[:, 0:KV * D]
            bvr = crow_sb[:, KV * D:2 * KV * D]
            bor = crow_sb[:, 256:256 + DIM]
            ind_sb = crow_sb[:, 1280:1280 + TH]

            wkv_big = wpool.tile([128, 8 * 2 * KV * D], bf16, tag="wkv")
            wkv_v = wkv_big.rearrange("p (k c) -> p k c", k=8)
            wk_sb = [wkv_v[:, k, 0:KV * D] for k in range(8)]
            wv_sb = [wkv_v[:, k, KV * D:2 * KV * D] for k in range(8)]
            xT_big = wpool.tile([128, 8 * TH], bf16, tag="xT")
            xT_v = xT_big.rearrange("p (k c) -> p k c", k=8)
            xT_sb = [xT_v[:, k, :] for k in range(8)]
            wq_big = wpool.tile([128, 8 * DIM], bf16, tag="wq")
            wq_v = wq_big.rearrange("p (k c) -> p k c", k=8)
            wq_sb = [wq_v[:, k, :] for k in range(8)]
            wo_big = wpool.tile([128, 8 * DIM], bf16, tag="wo")
            wo_v = wo_big.rearrange("p (k c) -> p k c", k=8)
            wo_sb = [wo_v[:, k, :] for k in range(8)]

            xT_r = xT.rearrange("(k p) c -> p k c", k=8)
            wq_r = wq.rearrange("(k p) c -> p k c", k=8)
            wo_r = wo.rearrange("(k p) c -> p k c", k=8)
            wkv_r = wkv.rearrange("(k p) c -> p k c", k=8)

            # Wq arrives as per-m column slices so each qT tile's projection
            # can start as soon as its own 256KB lands (overlaps the DMA).
            # The host stores Wq m-major ([m, p, k, c]) so each slice is one
            # fully contiguous 256KB read with 2KB per-partition lines.
            wq_mr = wq.rearrange("(m p) (k c) -> m p k c", m=8, k=8)

            def wq_m(eng, m):
                eng.dma_start(out=wq_v[:, :, m * 128:(m + 1) * 128],
                              in_=wq_mr[m])

            # sync queue: xT slab halves + wq m-slices + slab1 + wo (k 0-3)
            nc.sync.dma_start(out=xT_v[:, 0:4, 0:XSPLIT],
                              in_=xT_r[:, 0:4, 0:XSPLIT])
            wq_m(nc.sync, 0)
            wq_m(nc.sync, 2)
            wq_m(nc.sync, 4)
            wq_m(nc.sync, 6)
            nc.sync.dma_start(out=xT_v[:, 0:4, XSPLIT:TH],
                              in_=xT_r[:, 0:4, XSPLIT:TH])
            nc.sync.dma_start(out=wo_v[:, 0:4, :], in_=wo_r[:, 0:4, :])
            # scalar queue: the other halves
            nc.scalar.dma_start(out=xT_v[:, 4:8, 0:XSPLIT],
                                in_=xT_r[:, 4:8, 0:XSPLIT])
            wq_m(nc.scalar, 1)
            wq_m(nc.scalar, 3)
            wq_m(nc.scalar, 5)
            wq_m(nc.scalar, 7)
            nc.scalar.dma_start(out=xT_v[:, 4:8, XSPLIT:TH],
                                in_=xT_r[:, 4:8, XSPLIT:TH])
            nc.scalar.dma_start(out=wo_v[:, 4:8, :], in_=wo_r[:, 4:8, :])
            # gpsimd queue: small constants + wkv, then the mask build below
            ident = const.tile([128, 128], bf16, tag="ident")
            nc.gpsimd.dma_start(out=ident, in_=identd[:, :])
            nc.gpsimd.dma_start(out=crow_sb, in_=crow[:, :])
            nc.gpsimd.dma_start(out=bq_sb, in_=bqc[:, :])
            nc.gpsimd.dma_start(out=wkv_v[:, :, :], in_=wkv_r)

            # 0/1 window masks, transposed (key r, query c) orientation, for
            # the two triangular chunks; both kv-halves side by side.
            # j=0 chunk: valid where r >= c; j=2 chunk: valid where r < c.
            # Built on gpsimd AFTER its DMA issues (the engine is idle then).
            masks = const.tile([128, 2048], bf16, tag="masks")
            mask_lo = masks[:, 0:1024]
            mask_hi = masks[:, 1024:2048]
            nc.gpsimd.memset(mask_lo, 1.0)
            nc.gpsimd.memset(mask_hi, 1.0)
            for blk2 in range(0, 1024, 128):
                nc.gpsimd.affine_select(
                    out=mask_lo[:, blk2:blk2 + 128],
                    in_=mask_lo[:, blk2:blk2 + 128],
                    compare_op=mybir.AluOpType.is_ge,
                    fill=0.0, base=0, pattern=[[-1, 128]],
                    channel_multiplier=1)
                nc.gpsimd.affine_select(
                    out=mask_hi[:, blk2:blk2 + 128],
                    in_=mask_hi[:, blk2:blk2 + 128],
                    compare_op=mybir.AluOpType.is_ge,
                    fill=0.0, base=-1, pattern=[[1, 128]],
                    channel_multiplier=-1)

            # bo broadcast to all 128 partitions (K=1 outer product), used
            # by the DVE copy-out add; replaces per-tile K=1 bias matmuls.
            bo_bc = const.tile([128, DIM], bf16, tag="bo_bc")
            for n in range(2):
                bo_ps = ps.tile([128, 512], f32, tag="proj", name="bo_ps")
                nc.tensor.matmul(out=bo_ps, lhsT=ones_row,
                                 rhs=bor[:, n * 512:(n + 1) * 512],
                                 start=True, stop=True)
                nc.vector.tensor_copy(out=bo_bc[:, n * 512:(n + 1) * 512],
                                      in_=bo_ps)

            # ---- K projection over halo; zero at padded rows via ind fold -
            kT_sb = actp.tile([128, TH], bf16, tag="kT")

            def k_proj(c0, cw):
                k_ps = ps.tile([128, 512], f32, tag="proj", name="k_ps")
                for k in range(8):
                    nc.tensor.matmul(
                        out=k_ps[:, :cw], lhsT=wk_sb[k],
                        rhs=xT_sb[k][:, c0:c0 + cw],
                        start=(k == 0), stop=False)
                nc.tensor.matmul(
                    out=k_ps[:, :cw], lhsT=bkr, rhs=ind_sb[:, c0:c0 + cw],
                    start=False, stop=True)
                nc.vector.tensor_copy(out=kT_sb[:, c0:c0 + cw],
                                      in_=k_ps[:, :cw])

            # ---- V projection (keys on partitions). Layout per u-tile is
            # [V_kv0 (64) | 1 | V_kv1 (64) | 1]: the ones column appended to
            # each kv-slice makes the probs@[V|1] matmul emit the softmax
            # denominator as output column 64 for free. ---------------------
            NU = TH // 128
            v_sb = actp.tile([128, NU * 130], bf16, tag="V")
            v_view = v_sb.rearrange("p (u g c) -> p u g c", u=NU, g=2)
            nc.vector.memset(v_view[:, :, :, 64:65], 1.0)

            def v_proj(ut):
                v_ps = ps.tile([128, 512], f32, tag="proj", name="v_ps")
                for k in range(8):
                    nc.tensor.matmul(
                        out=v_ps[:, :KV * D],
                        lhsT=xT_sb[k][:, ut * 128:(ut + 1) * 128],
                        rhs=wv_sb[k], start=(k == 0), stop=False)
                nc.tensor.matmul(
                    out=v_ps[:, :KV * D],
                    lhsT=ind_sb[:, ut * 128:(ut + 1) * 128], rhs=bvr,
                    start=False, stop=True)
                nc.vector.tensor_copy(
                    out=v_view[:, ut, :, 0:64],
                    in_=v_ps[:, :KV * D].rearrange("p (g c) -> p g c", g=2))

            # ---- Q projection: qT tile g holds heads (2g, 2g+1) along the
            # free dim and heads (+8) on the upper partition half ------------
            qT_sb = []
            for g in range(2):
                t_qt = actp.tile([128, 4 * T], bf16, tag=f"qT{g}",
                                 name=f"qT{g}")
                qT_sb.append(t_qt)

            def q_proj(m, n):
                q_ps = ps.tile([128, 512], f32, tag="proj", name="q_ps")
                for k in range(8):
                    nc.tensor.matmul(
                        out=q_ps,
                        lhsT=wq_sb[k][:, m * 128:(m + 1) * 128],
                        rhs=xT_sb[k][:, HW + n * 512: HW + (n + 1) * 512],
                        start=(k == 0), stop=(k == 7))
                off = (m % 4) * T + n * 512
                nc.vector.tensor_scalar_add(
                    out=qT_sb[m // 4][:, off:off + 512], in0=q_ps,
                    scalar1=bq_sb[:, m:m + 1])

            # ---- pre-attention work (data-arrival ordered) -----------------
            k_proj(0, 512)
            k_proj(512, XSPLIT - 512)
            for ut in range(3):
                v_proj(ut)
            for m in range(8):
                q_proj(m, 0)

            # ---- attention + output transpose + (skewed) out-projection ---
            attnT = actp.tile([128, 8 * T], bf16, tag="attnT")
            attnT_v = attnT.rearrange("p (k t) -> p k t", k=8)

            def out_proj_half(mt, n):
                out_t = attnp.tile([128, 512], bf16, tag="outt")
                o2 = ps.tile([128, 512], f32, tag="proj", name="o2_ps")
                for k in range(8):
                    nc.tensor.matmul(
                        out=o2,
                        lhsT=attnT[:, k * T + mt * 128:
                                   k * T + (mt + 1) * 128],
                        rhs=wo_sb[k][:, n * 512:(n + 1) * 512],
                        start=(k == 0), stop=(k == 7))
                nc.vector.tensor_add(out=out_t, in0=o2,
                                     in1=bo_bc[:, n * 512:(n + 1) * 512])
                nc.sync.dma_start(
                    out=out[mt * 128:(mt + 1) * 128, n * 512:(n + 1) * 512],
                    in_=out_t)

            def out_proj(mt):
                out_proj_half(mt, 0)
                out_proj_half(mt, 1)

            # PE filler work per (tile, slot): slots 0/1 run between the two
            # attention head-groups (covering the exp latency with
            # independent matmuls), slot 2 after the transposes.
            fillers = {
                0: ([lambda: v_proj(3)], [lambda: v_proj(4)],
                    [lambda: k_proj(XSPLIT, 512)]),
                1: ([lambda: k_proj(XSPLIT + 512, TH - XSPLIT - 512)],
                    [lambda: v_proj(5)], [lambda: out_proj(0)]),
                2: ([lambda: q_proj(0, 1), lambda: q_proj(1, 1)],
                    [lambda: v_proj(6), lambda: q_proj(2, 1)],
                    [lambda: q_proj(3, 1), lambda: out_proj(1)]),
                3: ([lambda: q_proj(4, 1), lambda: q_proj(5, 1)],
                    [lambda: v_proj(7), lambda: q_proj(6, 1)],
                    [lambda: q_proj(7, 1), lambda: out_proj(2)]),
                4: ([lambda: out_proj_half(3, 0), lambda: v_proj(8)],
                    [lambda: out_proj_half(3, 1), lambda: v_proj(9)], []),
                5: ([lambda: out_proj_half(4, 0)],
                    [lambda: out_proj_half(4, 1)], []),
                6: ([lambda: out_proj_half(5, 0)],
                    [lambda: out_proj_half(5, 1)], []),
                7: ([lambda: out_proj_half(6, 0)],
                    [lambda: out_proj_half(6, 1)], []),
            }

            for mt in range(8):
                qcol = mt * 128
                u0 = qcol  # halo col of first attended key
                attn_t = attnp.tile([128, DIM], bf16, tag="attn")
                for gg in range(2):
                    qv = qT_sb[gg].rearrange("p (i t) -> p i t", i=4)
                    p2s = []
                    for j in range(3):
                        # both kv-halves of key-chunk j, row-packed into one
                        # 2-bank PSUM tile; four same-kv heads stream as one
                        # N=512 rhs per half.
                        s2 = ps.tile([128, 1024], f32, tag="s2", bufs=2,
                                     name="s2")
                        for half in range(2):
                            nc.tensor.matmul(
                                out=s2[:, half * 512:(half + 1) * 512],
                                lhsT=kT_sb[half * 64:(half + 1) * 64,
                                           u0 + j * 128:u0 + (j + 1) * 128],
                                rhs=qv[half * 64:(half + 1) * 64, :,
                                       qcol:qcol + 128],
                                start=True, stop=True,
                                tile_position=(64 * half, 0))
                        p2 = attnp.tile([128, 1024], bf16, tag="P", bufs=6,
                                        name="p2")
                        nc.scalar.activation(out=p2, in_=s2, func=Exp)
                        if j == 0:
                            nc.vector.tensor_mul(p2, p2, mask_lo)
                        elif j == 2:
                            nc.vector.tensor_mul(p2, p2, mask_hi)
                        p2s.append(p2)
                    for half in range(2):
                        # 4 heads share one PSUM bank: [a, 0:64]=attn out,
                        # [a, 64]=softmax denominator.
                        o4 = ps.tile([128, 260], f32, tag="o4", bufs=2,
                                     name="o4")
                        o4v = o4.rearrange("p (a c) -> p a c", a=4)
                        for a in range(4):
                            for j in range(3):
                                nc.tensor.matmul(
                                    out=o4v[:, a, :],
                                    lhsT=p2s[j][:, half * 512 + a * 128:
                                                half * 512 + (a + 1) * 128],
                                    rhs=v_view[:, mt + j, half, 0:65],
                                    start=(j == 0), stop=(j == 2))
                        rc4 = attnp.tile([128, 4], f32, tag="rc4", bufs=4,
                                         name="rc4")
                        nc.vector.reciprocal(out=rc4[:, :].unsqueeze(2),
                                             in_=o4v[:, :, 64:65])
                        hbase = (4 * gg + 8 * half) * 64
                        dst = attn_t[:, hbase:hbase + 256].rearrange(
                            "p (a d) -> p a d", a=4)
                        nc.vector.tensor_mul(
                            dst, o4v[:, :, 0:64],
                            rc4[:, :].unsqueeze(2).broadcast_to([128, 4, 64]))
                    for f in fillers[mt][gg]:
                        f()
                # transpose attn rows (t) x cols (hd) -> attnT k-tiles
                for g in range(3):
                    kcnt = 3 if g < 2 else 2
                    at_ps = ps.tile([128, 384], bf16, tag="proj", bufs=2,
                                    name="at_ps")
                    for jj in range(kcnt):
                        kk = g * 3 + jj
                        nc.tensor.matmul(
                            out=at_ps[:, jj * 128:(jj + 1) * 128],
                            lhsT=attn_t[:, kk * 128:(kk + 1) * 128],
                            rhs=ident, is_transpose=True,
                            start=(jj == 0), stop=(jj == kcnt - 1))
                    src = at_ps[:, :kcnt * 128].rearrange(
                        "p (j c) -> p j c", j=kcnt)
                    dst = attnT_v[:, g * 3:g * 3 + kcnt, qcol:qcol + 128]
                    nc.vector.tensor_copy(out=dst, in_=src)
                for f in fillers[mt][2]:
                    f()
            out_proj(7)

    nc.compile()
    return nc


def _host_prep(x, Wq, bq, Wk, bk, Wv, bv, Wo, bo):
    import ml_dtypes
    bf16 = ml_dtypes.bfloat16

    # permute Wq/bq columns so qT m-tile holds head m on partitions 0-63 and
    # head m+8 on partitions 64-127 (enables row-packed score matmuls)
    idx = np.empty(DIM, dtype=np.int64)
    for m in range(8):
        for j in range(128):
            h = m if j < 64 else m + 8
            idx[m * 128 + j] = h * D + (j % 64)
    wq_p = Wq[:, idx]
    # m-major layout: row m*128+p, col k*128+c holds Wq_perm[k*128+p, m*128+c]
    wq_p = np.ascontiguousarray(
        wq_p.reshape(8, 128, 8, 128).transpose(2, 1, 0, 3).reshape(
            DIM, DIM)).astype(bf16)
    bq_p = bq[idx].astype(np.float32).reshape(8, 128).T.copy()  # (128, 8)
    wkv_b = np.ascontiguousarray(
        np.concatenate([Wk, Wv], axis=1)).astype(bf16)
    wo_b = np.ascontiguousarray(Wo).astype(bf16)

    ident_h = np.eye(128, dtype=np.float32).astype(bf16)
    r, c = np.arange(128)[:, None], np.arange(128)[None, :]
    mask_h = np.concatenate(
        [np.tile((r >= c).astype(np.float32), (1, 8)),
         np.tile((r < c).astype(np.float32), (1, 8))], axis=1).astype(bf16)

    in_maps = []
    for c in range(NCORES):
        b, qt = c // QT, c % QT
        lo, hi = qt * T - HW, qt * T + T + HW
        xs = np.zeros((TH, DIM), dtype=np.float32)
        s0, s1 = max(lo, 0), min(hi, S)
        xs[s0 - lo:s1 - lo] = x[b, s0:s1]
        crow = np.zeros((1, 2560), dtype=np.float32)
        crow[0, 0:128] = bk
        crow[0, 128:256] = bv
        crow[0, 256:1280] = bo
        crow[0, 1280 + (s0 - lo):1280 + (s1 - lo)] = 1.0
        in_maps.append({
            "xT": np.ascontiguousarray(xs.T).astype(bf16),
            "Wq": wq_p, "Wkv": wkv_b, "Wo": wo_b,
            "bqc": bq_p, "crow": crow.astype(bf16),
            "identd": ident_h, "maskd": mask_h,
        })
    return in_maps


def kernel(x, Wq, bq, Wk, bk, Wv, bv, Wo, bo):
    from concourse.bass_utils import run_bass_kernel_spmd

    x, Wq, bq, Wk, bk, Wv, bv, Wo, bo = (
        np.asarray(a, dtype=np.float32)
        for a in (x, Wq, bq, Wk, bk, Wv, bv, Wo, bo))
    nc = _build_nc()
    in_maps = _host_prep(x, Wq, bq, Wk, bk, Wv, bv, Wo, bo)
    res = run_bass_kernel_spmd(nc, in_maps, core_ids=list(range(NCORES)))
    out = np.empty((B, S, DIM), dtype=np.float32)
    for c in range(NCORES):
        b, qt = c // QT, c % QT
        out[b, qt * T:(qt + 1) * T] = res.results[c]["out"].astype(np.float32)
    return out
